# revision 1
# baseline (speedup 1.0000x reference)
"""CNLinkPredictor Trainium2 kernel.

Edge-sharded across 8 NeuronCores (1024 target edges each); x, adj, and the
MLP weights are replicated. Per core:
  A) h = x + MLP(x) computed in transposed layout: the host supplies xT, so
     stage A is matmul-only on PE (bf16, N=512 moving), fused bias+ReLU on
     the scalar engine, residual on DVE, then xbar DMA-transposes write h
     back to natural layout (bf16, (half, ktile, c) column order so every
     transpose destination is a contiguous per-partition span).
  B) per 128-edge block and k-half: indirect-DMA gather of the two adjacency
     rows per edge (fp8 - exact for a 0/1 adjacency - one row per SBUF
     partition), DVE multiply -> cn (bf16, still exact), one xbar
     DMA-transpose, then 32 matmuls accumulating cnT @ h into PSUM.
  C) edge MLPs in transposed layout (bf16, N=512 over 4-block groups), xbar
     transposes for xcn and xi*xj, final [1, 1024] output row.

Emission is software-pipelined (A first half, B k-half 0, A second half,
B k-half 1, C, ...) so the FIFO engine queues never head-of-line block on
data that is not ready yet.

Hardware pitfalls this kernel works around:
  - This walrus build accepts at most ONE sync-wait per instruction
    (_apply_tile_patch splits the Tile tail drain; _split_multi_waits hoists
    extra waits onto same-engine NoOps).
  - Concurrent 4-byte DMA traffic corrupts in-flight 2-byte xbar
    DMA-transposes, so every steady-state transfer is <= 2 bytes/element
    (fp8 adjacency, bf16 everything else); the few f32/int32 loads happen
    up front and the single f32 store happens after the last transpose.
  - xbar transposes into non-contiguous destinations produce wrong data;
    all transpose targets collapse to contiguous 2-D access patterns.
"""

import numpy as np
import ml_dtypes

N = 8192
C = 256
E = 8192
NCORES = 8
EL = E // NCORES          # edges per core
P = 128
NB = EL // P              # edge blocks per core
KH = 2                    # k halves for adjacency gather
KC = N // KH              # columns per half
NKT = N // P              # 64 k tiles
AGRP = 512                # stage-A node group
CGRP = 4                  # stage-C blocks per group (512 edges)

_CACHE = {}
TRACE = False
LAST_RESULT = None
DEBUG_DUMPS = False


def _apply_tile_patch():
    """Split the Tile tail-drain's multi-sem wait onto individual SP nops."""
    from concourse.tile import TileContext
    from concourse.vector_clock import ScopedClock

    if getattr(TileContext, "_drain_patched", False):
        return

    def _patched(self, tick_clock, wait_clock):
        nc = self.nc
        collector = nc.sync.nop()
        wait_clock.add_sem_waits(
            collector.ins, ScopedClock({None: tick_clock.global_clock})
        )
        si = collector.ins.sync_info
        waits = list(si.on_wait) if si is not None and si.on_wait else []
        if si is not None and len(waits) > 1:
            name_to_handle = {h.name: h for h in self.sems.allocated().values()}
            si.on_wait = [waits[0]]
            for w in waits[1:]:
                op = {
                    "sem-ge-imm": "sem-ge",
                    "sem-eq-imm": "sem-eq",
                    "sem-le-imm": "sem-le",
                }.get(str(w.wait_mode), "sem-ge")
                nc.sync.nop().wait_op(name_to_handle[w.ant_name], w.wait_value, op)
        nc.sync.drain()
        nc.all_engine_barrier()
        assert self.sems is not None
        popped = nc._tile_sem_poison_stack.pop()
        assert popped is self._sem_poison
        nc.clear_and_free_semaphores(list(self.sems.allocated().values()))
        nc.all_engine_barrier()

    TileContext._drain_and_barrier = _patched
    TileContext._drain_patched = True


def _split_multi_waits(nc):
    """Hoist extra sync-waits onto same-engine NoOps (sequential waits ==
    ANDed waits); this walrus build allows one wait per instruction."""
    import concourse.mybir as mybir

    cnt = 0
    for fn in nc.m.functions:
        for bb in fn.blocks:
            out = []
            for inst in bb.instructions:
                si = getattr(inst, "sync_info", None)
                waits = list(si.on_wait) if si is not None and si.on_wait else []
                if len(waits) > 1:
                    for w in waits[:-1]:
                        nop = mybir.InstNoOp(name=f"ws-{cnt}", ins=[], outs=[])
                        cnt += 1
                        nop.engine = inst.engine
                        nop.sync_info = mybir.SyncInfo(on_wait=[w], on_update=[])
                        out.append(nop)
                    si.on_wait = [waits[-1]]
                out.append(inst)
            bb.instructions = out
    return nc


def _build(split_waits=True):
    import concourse.bass as bass
    import concourse.mybir as mybir
    from concourse.tile import TileContext

    _apply_tile_patch()

    f32 = mybir.dt.float32
    f32r = mybir.dt.float32r
    bf16 = mybir.dt.bfloat16
    fp8 = mybir.dt.float8e4
    i32 = mybir.dt.int32
    Relu = mybir.ActivationFunctionType.Relu
    Ident = mybir.ActivationFunctionType.Identity
    MUL = mybir.AluOpType.mult
    ADD = mybir.AluOpType.add

    nc = bass.Bass(num_swdge_queues=4)

    xT_d = nc.dram_tensor("xT", [C, N], bf16, kind="ExternalInput")
    x_d = nc.dram_tensor("x", [N, C], bf16, kind="ExternalInput")
    adj_d = nc.dram_tensor("adj", [N, N], fp8, kind="ExternalInput")
    idx_d = nc.dram_tensor("idx", [2, EL], i32, kind="ExternalInput")
    # all matmul weights in bf16 (2-byte rule; see module docstring)
    wA = {n: nc.dram_tensor(n, [C, C], bf16, kind="ExternalInput")
          for n in ("xlin_w1", "xlin_w2")}
    wC = {n: nc.dram_tensor(n, [C, C], bf16, kind="ExternalInput")
          for n in ("xcn_w1", "xcn_w2", "xij_w", "lin_w1")}
    lin_w2_d = nc.dram_tensor("lin_w2", [C, 1], bf16, kind="ExternalInput")
    bnames = ["xlin_b1", "xlin_b2", "xcn_b1", "xcn_b2", "xij_b", "lin_b1"]
    ball_d = nc.dram_tensor("ball", [P, 2 * len(bnames)], f32,
                            kind="ExternalInput")
    lin_b2_d = nc.dram_tensor("lin_b2", [1, 1], f32, kind="ExternalInput")
    beta_d = nc.dram_tensor("beta_bc", [P, 1], f32, kind="ExternalInput")
    out_d = nc.dram_tensor("out", [1, EL], f32, kind="ExternalOutput")
    dbg = {}
    if DEBUG_DUMPS:
        dbg["h_all"] = nc.dram_tensor("dbg_h", [P, 2 * N], bf16,
                                      kind="ExternalOutput")
        dbg["cn"] = nc.dram_tensor("dbg_cn", [P, KC], bf16,
                                   kind="ExternalOutput")
        dbg["cnT"] = nc.dram_tensor("dbg_cnT", [P, KC], bf16,
                                    kind="ExternalOutput")
        dbg["xcn"] = nc.dram_tensor("dbg_xcn", [P, C], bf16,
                                    kind="ExternalOutput")
        dbg["xcnT"] = nc.dram_tensor("dbg_xcnT", [P, 2 * CGRP * P], bf16,
                                     kind="ExternalOutput")
        dbg["prodT"] = nc.dram_tensor("dbg_prodT", [P, 2 * CGRP * P], bf16,
                                      kind="ExternalOutput")

    _swq = [0]

    def _rr(inst):
        q = _swq[0] % 4
        _swq[0] += 1
        if q:
            inst.ins.queue = f"qPoolDynamic{q}"
        return inst

    with TileContext(nc) as tc:
        with (
            tc.tile_pool(name="const", bufs=1) as pK,
            tc.tile_pool(name="hpool", bufs=1) as pH,
            tc.tile_pool(name="adj", bufs=5) as pAdj,
            tc.tile_pool(name="cn", bufs=4) as pCn,
            tc.tile_pool(name="cnT", bufs=4) as pT,
            tc.tile_pool(name="edge", bufs=2) as pC,
            tc.tile_pool(name="xcn", bufs=CGRP) as pX,
        ):
            # ---- constants ----
            # idx first: the stage-B gathers depend only on these
            idx_sb = pK.tile([P, 2 * NB], i32, tag="idx_sb", name="idx_sb")
            nc.sync.dma_start(
                out=idx_sb[:].rearrange("p (t b) -> p t b", t=2),
                in_=idx_d[:, :].rearrange("t (b p) -> p t b", p=P),
            )
            ii = [idx_sb[:, b:b + 1] for b in range(NB)]
            jj = [idx_sb[:, NB + b:NB + b + 1] for b in range(NB)]

            wA_sb, wC_sb = {}, {}
            for n, t_d in list(wA.items()) + list(wC.items()):
                t = pK.tile([P, 2 * C], bf16, tag=f"w_{n}", name=f"w_{n}")
                nc.sync.dma_start(
                    out=t[:].rearrange("p (k n2) -> p k n2", k=2),
                    in_=t_d[:, :].rearrange("(k p) n2 -> p k n2", p=P),
                )
                pair = [t[:, 0:C], t[:, C:2 * C]]
                (wA_sb if n in wA else wC_sb)[n] = pair
            lw2_t = pK.tile([P, 2], bf16, tag="lin_w2", name="lin_w2t")
            nc.sync.dma_start(
                out=lw2_t[:].rearrange("p (k o) -> p k o", k=2),
                in_=lin_w2_d[:, :].rearrange("(k p) o -> p k o", p=P),
            )
            lw2_sb = [lw2_t[:, 0:1], lw2_t[:, 1:2]]
            b_sb = {}
            ball = pK.tile([P, 2 * len(bnames)], f32, tag="ball", name="ball")
            nc.sync.dma_start(
                out=ball[:],
                in_=ball_d[:, :],
            )
            for q, n in enumerate(bnames):
                b_sb[n] = ball[:, 2 * q:2 * q + 2]
            lb2_sb = pK.tile([1, 1], f32, tag="b_lin2", name="b_lin2")
            nc.sync.dma_start(out=lb2_sb[:], in_=lin_b2_d[:, :])
            beta_sb = pK.tile([P, 1], f32, tag="beta", name="beta")
            nc.sync.dma_start(out=beta_sb[:], in_=beta_d[:, :])

            out_row = pK.tile([1, EL], f32, tag="out_row", name="out_row")
            # natural-layout h in (hh, kt, c2) order so the xbar transposes
            # write contiguous per-partition spans: column = hh*N + kt*128 + c2
            # encodes h[node = kt*128 + p, channel = hh*128 + c2].
            h_all = pH.tile([P, 2 * N], bf16, tag="h_all", name="h_all")
            h_view = h_all[:].rearrange("p (hh kt c) -> p hh kt c", hh=2, c=P)

            # ---- stage definitions ----
            def stage_a_group(g, pA, psA):
                m0 = g * AGRP
                xT = []
                for h in range(2):
                    t = pA.tile([P, AGRP], bf16, tag=f"xT{h}", name=f"xT{h}_{g}")
                    nc.scalar.dma_start(
                        out=t[:], in_=xT_d[h * P:(h + 1) * P, m0:m0 + AGRP]
                    )
                    xT.append(t)
                y1T = []
                for h in range(2):
                    ps = psA.tile([P, AGRP], f32, tag="psmm", name=f"psA1_{g}{h}")
                    nc.tensor.matmul(
                        ps[:], wA_sb["xlin_w1"][0][:, h * P:(h + 1) * P],
                        xT[0][:], start=True, stop=False,
                    )
                    nc.tensor.matmul(
                        ps[:], wA_sb["xlin_w1"][1][:, h * P:(h + 1) * P],
                        xT[1][:], start=False, stop=True,
                    )
                    t = pA.tile([P, AGRP], bf16, tag=f"y1T{h}", name=f"y1T{h}_{g}")
                    nc.scalar.activation(
                        t[:], ps[:], Relu, bias=b_sb["xlin_b1"][:, h:h + 1]
                    )
                    y1T.append(t)
                for h in range(2):
                    ps = psA.tile([P, AGRP], f32, tag="psmm", name=f"psA2_{g}{h}")
                    nc.tensor.matmul(
                        ps[:], wA_sb["xlin_w2"][0][:, h * P:(h + 1) * P],
                        y1T[0][:], start=True, stop=False,
                    )
                    nc.tensor.matmul(
                        ps[:], wA_sb["xlin_w2"][1][:, h * P:(h + 1) * P],
                        y1T[1][:], start=False, stop=True,
                    )
                    y2 = pA.tile([P, AGRP], bf16, tag="y2T", name=f"y2T{h}_{g}")
                    nc.scalar.activation(
                        y2[:], ps[:], Relu, bias=b_sb["xlin_b2"][:, h:h + 1]
                    )
                    hT = pA.tile([P, AGRP], bf16, tag=f"hT{h}", name=f"hT{h}_{g}")
                    nc.vector.tensor_tensor(
                        out=hT[:], in0=xT[h][:], in1=y2[:], op=ADD
                    )
                    nc.sync.dma_start_transpose(
                        out=h_view[:, h,
                                   g * (AGRP // P):(g + 1) * (AGRP // P), :],
                        in_=hT[:],
                    )

            xcn_sb = [None] * NB

            cnT_map = {}

            def stage_b_load(b, s):
                ai = pAdj.tile([P, KC], fp8, tag="ai", name=f"ai{b}_{s}")
                _rr(nc.gpsimd.indirect_dma_start(
                    out=ai[:], out_offset=None, in_=adj_d[:, :],
                    in_offset=bass.IndirectOffsetOnAxis(ap=ii[b][:, :1], axis=0),
                    element_offset=s * KC,
                ))
                aj = pAdj.tile([P, KC], fp8, tag="aj", name=f"aj{b}_{s}")
                _rr(nc.gpsimd.indirect_dma_start(
                    out=aj[:], out_offset=None, in_=adj_d[:, :],
                    in_offset=bass.IndirectOffsetOnAxis(ap=jj[b][:, :1], axis=0),
                    element_offset=s * KC,
                ))
                cn = pCn.tile([P, KC], bf16, tag="cn", name=f"cn{b}_{s}")
                nc.vector.tensor_tensor(out=cn[:], in0=ai[:], in1=aj[:], op=MUL)
                cnT = pT.tile([P, KC], bf16, tag="cnT", name=f"cnT{b}_{s}")
                nc.sync.dma_start_transpose(
                    out=cnT[:].rearrange("p (kt e) -> p kt e", e=P),
                    in_=cn[:],
                )
                cnT_map[(b, s)] = cnT

            def stage_b_mms(b, s, psxcn):
                cnT = cnT_map[(b, s)]
                for kt in range(KC // P):
                    ktg = s * (KC // P) + kt
                    nc.tensor.matmul(
                        psxcn[:],
                        cnT[:, kt * P:(kt + 1) * P],
                        h_view[:, :, ktg, :],
                        start=(ktg == 0), stop=(ktg == NKT - 1),
                    )

            def stage_b_finish(b, psxcn):
                xcn_sb[b] = pX.tile([P, C], bf16, tag="xcn", name=f"xcn{b}")
                nc.vector.tensor_copy(xcn_sb[b][:], psxcn[:])

            prodT_map = {}

            def stage_c_prod(grp):
                blocks = range(grp * CGRP, (grp + 1) * CGRP)
                W = CGRP * P
                prodT = pC.tile([P, 2 * W], bf16, tag="prodT", name=f"prodT{grp}")
                prodT_v = prodT[:].rearrange(
                    "p (blk hh e) -> p blk hh e", blk=CGRP, e=P)
                prodT_map[grp] = prodT
                for t2, b in enumerate(blocks):
                    xi = pC.tile([P, C], bf16, tag="xi", name=f"xi{b}")
                    _rr(nc.gpsimd.indirect_dma_start(
                        out=xi[:], out_offset=None, in_=x_d[:, :],
                        in_offset=bass.IndirectOffsetOnAxis(
                            ap=ii[b][:, :1], axis=0),
                    ))
                    xj = pC.tile([P, C], bf16, tag="xj", name=f"xj{b}")
                    _rr(nc.gpsimd.indirect_dma_start(
                        out=xj[:], out_offset=None, in_=x_d[:, :],
                        in_offset=bass.IndirectOffsetOnAxis(
                            ap=jj[b][:, :1], axis=0),
                    ))
                    pt = pC.tile([P, C], bf16, tag="prod", name=f"prod{b}")
                    nc.vector.tensor_tensor(
                        out=pt[:], in0=xi[:], in1=xj[:], op=MUL
                    )
                    nc.sync.dma_start_transpose(
                        out=prodT_v[:, t2, :, :], in_=pt[:],
                    )

            def stage_c(grp, psC, psO):
                blocks = range(grp * CGRP, (grp + 1) * CGRP)
                W = CGRP * P  # 512 edges
                xcnT = pC.tile([P, 2 * W], bf16, tag="xcnT", name=f"xcnT{grp}")
                xcnT_v = xcnT[:].rearrange(
                    "p (blk hh e) -> p blk hh e", blk=CGRP, e=P)
                prodT = prodT_map[grp]
                for t2, b in enumerate(blocks):
                    nc.sync.dma_start_transpose(
                        out=xcnT_v[:, t2, :, :], in_=xcn_sb[b][:],
                    )

                def mlp_layer(rhs2, wname, bname, outtag, packed):
                    outs = []
                    for h in range(2):
                        ps = psC.tile([P, W], f32, tag="psc",
                                      name=f"psc_{grp}_{outtag}{h}")
                        if packed:
                            rhs_v = rhs2[:].rearrange(
                                "p (blk hh e) -> p blk hh e", blk=CGRP, e=P)
                            r0, r1 = rhs_v[:, :, 0, :], rhs_v[:, :, 1, :]
                        else:
                            r0, r1 = rhs2[0][:], rhs2[1][:]
                        nc.tensor.matmul(
                            ps[:], wC_sb[wname][0][:, h * P:(h + 1) * P],
                            r0, start=True, stop=False,
                        )
                        nc.tensor.matmul(
                            ps[:], wC_sb[wname][1][:, h * P:(h + 1) * P],
                            r1, start=False, stop=True,
                        )
                        t = pC.tile([P, W], bf16, tag=f"{outtag}{h}",
                                    name=f"{outtag}{h}_{grp}")
                        nc.scalar.activation(
                            t[:], ps[:], Relu, bias=b_sb[bname][:, h:h + 1]
                        )
                        outs.append(t)
                    return outs

                xijT = mlp_layer(prodT, "xij_w", "xij_b", "xijT", True)
                u1T = mlp_layer(xcnT, "xcn_w1", "xcn_b1", "u1T", True)
                u2T = mlp_layer(u1T, "xcn_w2", "xcn_b2", "u2T", False)
                zT = []
                for h in range(2):
                    zb = pC.tile([P, W], bf16, tag=f"zb{h}", name=f"zb{h}_{grp}")
                    nc.vector.tensor_tensor(
                        out=zb[:], in0=u2T[h][:],
                        in1=beta_sb[:, 0:1].to_broadcast([P, W]), op=MUL,
                    )
                    zt = pC.tile([P, W], bf16, tag=f"zT{h}", name=f"zT{h}_{grp}")
                    nc.vector.tensor_tensor(
                        out=zt[:], in0=zb[:], in1=xijT[h][:], op=ADD
                    )
                    zT.append(zt)
                vT = mlp_layer(zT, "lin_w1", "lin_b1", "vT", False)
                pso = psO.tile([1, W], f32, tag="pso", name=f"pso{grp}")
                nc.tensor.matmul(
                    pso[:], lw2_sb[0][:], vT[0][:], start=True, stop=False
                )
                nc.tensor.matmul(
                    pso[:], lw2_sb[1][:], vT[1][:], start=False, stop=True
                )
                nc.scalar.activation(
                    out_row[0:1, grp * W:(grp + 1) * W], pso[:],
                    Ident, bias=lb2_sb[0:1, 0:1],
                )

            # ---- software-pipelined emission ----
            with tc.tile_pool(name="psB", bufs=1, space="PSUM") as psB:
                ps_map = {}

                def open_half(bh):
                    for b in range(bh * CGRP, (bh + 1) * CGRP):
                        ps_map[b] = psB.tile(
                            [P, C], f32, tag=f"psxcn{b % CGRP}",
                            name=f"psxcn{b}")

                def b_loads(bh, s):
                    for b in range(bh * CGRP, (bh + 1) * CGRP):
                        stage_b_load(b, s)

                def b_mms(bh, s):
                    for b in range(bh * CGRP, (bh + 1) * CGRP):
                        stage_b_mms(b, s, ps_map[b])

                with tc.tile_pool(name="stA", bufs=3) as pA, \
                     tc.tile_pool(name="psA", bufs=4, space="PSUM") as psA:
                    open_half(0)
                    b_loads(0, 0)
                    for g in range(8):
                        stage_a_group(g, pA, psA)
                    b_mms(0, 0)
                    stage_c_prod(0)
                    b_loads(0, 1)
                    for g in range(8, 16):
                        stage_a_group(g, pA, psA)
                with tc.tile_pool(name="psC", bufs=2, space="PSUM") as psC, \
                     tc.tile_pool(name="psO", bufs=1, space="PSUM") as psO:
                    b_mms(0, 1)
                    stage_c_prod(1)
                    for b in range(CGRP):
                        stage_b_finish(b, ps_map[b])
                    open_half(1)
                    b_loads(1, 0)
                    b_mms(1, 0)
                    b_loads(1, 1)
                    stage_c(0, psC, psO)
                    b_mms(1, 1)
                    for b in range(CGRP, 2 * CGRP):
                        stage_b_finish(b, ps_map[b])
                    stage_c(1, psC, psO)

            nc.sync.dma_start(out=out_d[:, :], in_=out_row[0:1, :])
            if DEBUG_DUMPS:
                nc.sync.dma_start(out=dbg["h_all"][:, :], in_=h_all[:])

    return _split_multi_waits(nc) if split_waits else nc


def kernel(**inputs):
    from concourse.bass_utils import run_bass_kernel_spmd

    if "nc" not in _CACHE:
        _CACHE["nc"] = _build()
    nc = _CACHE["nc"]

    x = np.ascontiguousarray(inputs["x"], dtype=np.float32)
    adj8 = np.ascontiguousarray(inputs["adj"]).astype(ml_dtypes.float8_e4m3)
    tar = np.asarray(inputs["tar_ei"]).astype(np.int32)

    def btile(b):
        return np.ascontiguousarray(np.asarray(b, dtype=np.float32).reshape(2, P).T)

    common = {
        "x": x.astype(ml_dtypes.bfloat16),
        "xT": np.ascontiguousarray(x.T).astype(ml_dtypes.bfloat16),
        "adj": adj8,
        "beta_bc": np.full((P, 1), np.asarray(inputs["beta"]).reshape(-1)[0],
                           dtype=np.float32),
        "lin_w2": np.ascontiguousarray(inputs["lin_w2"]).astype(ml_dtypes.bfloat16),
        "lin_b2": np.asarray(inputs["lin_b2"], dtype=np.float32).reshape(1, 1),
    }
    for n in ("xlin_w1", "xlin_w2"):
        common[n] = np.ascontiguousarray(inputs[n]).astype(ml_dtypes.bfloat16)
    for n in ("xcn_w1", "xcn_w2", "xij_w", "lin_w1"):
        common[n] = np.ascontiguousarray(inputs[n]).astype(ml_dtypes.bfloat16)
    common["ball"] = np.ascontiguousarray(np.concatenate(
        [btile(inputs[n]) for n in
         ("xlin_b1", "xlin_b2", "xcn_b1", "xcn_b2", "xij_b", "lin_b1")],
        axis=1))

    in_maps = []
    for c in range(NCORES):
        m = dict(common)
        m["idx"] = np.ascontiguousarray(tar[:, c * EL:(c + 1) * EL])
        in_maps.append(m)

    res = run_bass_kernel_spmd(
        nc, in_maps, core_ids=list(range(NCORES)), trace=TRACE
    )
    global LAST_RESULT
    LAST_RESULT = res
    out = np.concatenate(
        [res.results[c]["out"].reshape(EL, 1) for c in range(NCORES)], axis=0
    )
    return out.astype(np.float32)



# revision 19
# speedup vs baseline: 1.0293x; 1.0293x over previous
"""CNLinkPredictor Trainium2 kernel.

Edge-sharded across 8 NeuronCores (1024 target edges each); x, adj, and the
MLP weights are replicated. Per core:
  A) h = x + MLP(x) in transposed layout: host supplies a column-PERMUTED xT
     (see below), so stage A is matmul-only on PE (bf16, N=512 moving), fused
     bias+ReLU on the scalar engine, residual on DVE, then xbar DMA-transposes
     write h back to natural layout.
  B) per 128-edge block: indirect-DMA gather of the two FULL adjacency rows
     per edge (fp8 - exact for a 0/1 adjacency), cn = ai AND aj as a bitwise
     AND on uint16 views (fp8 pairs; 0/1 fp8 codes AND exactly), one xbar
     DMA-transpose of the uint16 view (halves transpose cost vs bf16), then
     FLIPPED matmuls: lhsT = h k-tile (bf16), rhs = cnT for all 512 edges of
     an edge-group (fp8, stride-2 AP) accumulating xcnT[c,e] in PSUM.
     The uint16-pair transpose leaves k-pairs interleaved: partition p of
     k-tile tt holds k = 256*tt + 2p + par. The host permutes xT columns so
     h_view tile ktg=2*tt+par, partition p holds node 256*tt+2p+par; the
     device math is then an exact relabeling (verified vs reference).
  C) edge MLPs in transposed layout per 512-edge group; xcnT comes straight
     out of the stage-B PSUM (no transpose), xi*xj is transposed per block.

PE order A -> B(edges 0-511) -> C(0-511) -> B(512-1023) -> C(512-1023) keeps
the tensor engine hot; DVE ANDs and xbar transposes are interleaved into the
stage-A emission so the FIFO queues never head-of-line block.

Hardware pitfalls this kernel works around:
  - This walrus build accepts at most ONE sync-wait per instruction
    (_apply_tile_patch splits the Tile tail drain; _split_multi_waits hoists
    extra waits onto same-engine NoOps).
  - Concurrent 4-byte DMA traffic corrupts in-flight 2-byte xbar
    DMA-transposes, so every steady-state transfer is <= 2 bytes/element
    (fp8 adjacency, bf16/u16 everything else); the few f32/int32 loads happen
    up front and the single f32 store happens after the last transpose.
  - xbar transposes into non-contiguous destinations produce wrong data;
    all transpose targets collapse to contiguous 2-D access patterns.
"""

import numpy as np
import ml_dtypes

N = 8192
C = 256
E = 8192
NCORES = 8
EL = E // NCORES          # edges per core
P = 128
NB = EL // P              # edge blocks per core (8)
TT = N // 256             # k-pair tiles (32): tile tt holds k=256*tt+2p+par
AGRP = 512                # stage-A node group
NG = N // AGRP            # stage-A groups (16)
CGRP = 4                  # blocks per edge-group (512 edges)
NEG = NB // CGRP          # edge groups (2)
W = CGRP * P              # edges per group (512)

_CACHE = {}
TRACE = False
LAST_RESULT = None
_HW_LANES = {}            # instruction name -> pinned DMAHW sem lane
_SW_LANES = {}            # instruction name -> pinned DMASW sem lane
CFG = {
    "lane_cn": 7,          # DMAHW lane for cn transposes (None = default rr)
    "lane_const": None,    # lane for const loads
    "lane_prod": None,     # lane for prodT xposes + out store
    "lane_xT": False,      # xT loads on lanes 0/1, hT xposes on 2/3
    "sw_xixj": False,      # xi/xj on SW lanes 6/7
    "hipri_adj": False,    # high_priority on gathers + and/xpose
    "inplace_and": True,   # AND writes into ai tile (frees cn pool)
}


def _apply_tile_patch():
    """Split the Tile tail-drain's multi-sem wait onto individual SP nops."""
    from concourse.tile import TileContext
    from concourse.vector_clock import ScopedClock

    if getattr(TileContext, "_drain_patched", False):
        return

    def _patched(self, tick_clock, wait_clock):
        nc = self.nc
        collector = nc.sync.nop()
        wait_clock.add_sem_waits(
            collector.ins, ScopedClock({None: tick_clock.global_clock})
        )
        si = collector.ins.sync_info
        waits = list(si.on_wait) if si is not None and si.on_wait else []
        if si is not None and len(waits) > 1:
            name_to_handle = {h.name: h for h in self.sems.allocated().values()}
            si.on_wait = [waits[0]]
            for w in waits[1:]:
                op = {
                    "sem-ge-imm": "sem-ge",
                    "sem-eq-imm": "sem-eq",
                    "sem-le-imm": "sem-le",
                }.get(str(w.wait_mode), "sem-ge")
                nc.sync.nop().wait_op(name_to_handle[w.ant_name], w.wait_value, op)
        nc.sync.drain()
        nc.all_engine_barrier()
        assert self.sems is not None
        popped = nc._tile_sem_poison_stack.pop()
        assert popped is self._sem_poison
        nc.clear_and_free_semaphores(list(self.sems.allocated().values()))
        nc.all_engine_barrier()

    TileContext._drain_and_barrier = _patched
    TileContext._drain_patched = True


def _apply_lane_patch():
    """Honor a `_hw_lane` attribute on DMA instructions: pin them to that
    DMAHW sem lane instead of the global round-robin, and keep the round-robin
    off the reserved lanes. Without this, slow gather-gated transposes share a
    lane counter with fast stage-A DMAs and unrelated deps summarize onto the
    slow stream (multi-10us head-of-line stalls)."""
    from concourse.tile_sem_assignment import TileClockTick

    if getattr(TileClockTick, "_lane_patched", False):
        return
    orig = TileClockTick._assign_tick

    def _patched(self, inst):
        name = getattr(inst, "name", None)
        hw = _HW_LANES.get(name)
        if hw is not None:
            saved = self.next_hw_dma_idx
            self.next_hw_dma_idx = hw
            orig(self, inst)
            self.next_hw_dma_idx = saved
            return
        sw = _SW_LANES.get(name)
        if sw is not None:
            saved = self.next_sw_dma_idx
            self.next_sw_dma_idx = sw
            orig(self, inst)
            self.next_sw_dma_idx = saved
            return
        orig(self, inst)
        rhw = set(_HW_LANES.values())
        rsw = set(_SW_LANES.values())
        if len(rhw) < 8:
            while self.next_hw_dma_idx in rhw:
                self.next_hw_dma_idx = (self.next_hw_dma_idx + 1) % 8
        if len(rsw) < self.swdge_sem_count:
            while self.next_sw_dma_idx in rsw:
                self.next_sw_dma_idx = (self.next_sw_dma_idx + 1) % self.swdge_sem_count

    TileClockTick._assign_tick = _patched
    TileClockTick._lane_patched = True


def _split_multi_waits(nc):
    """Hoist extra sync-waits onto same-engine NoOps (sequential waits ==
    ANDed waits); this walrus build allows one wait per instruction."""
    import concourse.mybir as mybir

    cnt = 0
    for fn in nc.m.functions:
        for bb in fn.blocks:
            out = []
            for inst in bb.instructions:
                si = getattr(inst, "sync_info", None)
                waits = list(si.on_wait) if si is not None and si.on_wait else []
                if len(waits) > 1:
                    for w in waits[:-1]:
                        nop = mybir.InstNoOp(name=f"ws-{cnt}", ins=[], outs=[])
                        cnt += 1
                        nop.engine = inst.engine
                        nop.sync_info = mybir.SyncInfo(on_wait=[w], on_update=[])
                        out.append(nop)
                    si.on_wait = [waits[-1]]
                out.append(inst)
            bb.instructions = out
    return nc


def _build(split_waits=True):
    import concourse.bass as bass
    import concourse.mybir as mybir
    from concourse.tile import TileContext

    _apply_tile_patch()
    _apply_lane_patch()
    _HW_LANES.clear()
    _SW_LANES.clear()

    f32 = mybir.dt.float32
    bf16 = mybir.dt.bfloat16
    fp8 = mybir.dt.float8e4
    u16 = mybir.dt.uint16
    i32 = mybir.dt.int32
    Relu = mybir.ActivationFunctionType.Relu
    Ident = mybir.ActivationFunctionType.Identity
    MUL = mybir.AluOpType.mult
    ADD = mybir.AluOpType.add
    AND = mybir.AluOpType.bitwise_and

    nc = bass.Bass(num_swdge_queues=4)

    xT_d = nc.dram_tensor("xTp", [C, N], bf16, kind="ExternalInput")
    x_d = nc.dram_tensor("x", [N, C], bf16, kind="ExternalInput")
    adj_d = nc.dram_tensor("adj", [N, N], fp8, kind="ExternalInput")
    idx_d = nc.dram_tensor("idx", [2, EL], i32, kind="ExternalInput")
    wA = {n: nc.dram_tensor(n, [C, C], bf16, kind="ExternalInput")
          for n in ("xlin_w1", "xlin_w2")}
    wC = {n: nc.dram_tensor(n, [C, C], bf16, kind="ExternalInput")
          for n in ("xcn_w1", "xcn_w2", "xij_w", "lin_w1")}
    lin_w2_d = nc.dram_tensor("lin_w2", [C, 1], bf16, kind="ExternalInput")
    bnames = ["xlin_b1", "xlin_b2", "xcn_b1", "xcn_b2", "xij_b", "lin_b1"]
    ball_d = nc.dram_tensor("ball", [P, 2 * len(bnames)], f32,
                            kind="ExternalInput")
    lin_b2_d = nc.dram_tensor("lin_b2", [1, 1], f32, kind="ExternalInput")
    beta_d = nc.dram_tensor("beta_bc", [P, 1], f32, kind="ExternalInput")
    out_d = nc.dram_tensor("out", [1, EL], f32, kind="ExternalOutput")

    _swq = [0]

    def _rr(inst):
        q = _swq[0] % 4
        _swq[0] += 1
        if q:
            inst.ins.queue = f"qPoolDynamic{q}"
        return inst

    with TileContext(nc) as tc:
        with (
            tc.tile_pool(name="const", bufs=1) as pK,
            tc.tile_pool(name="hpool", bufs=1) as pH,
            tc.tile_pool(name="adji", bufs=3) as pAdjI,
            tc.tile_pool(name="adjj", bufs=2) as pAdjJ,
            tc.tile_pool(name="cnT", bufs=1) as pT,
            tc.tile_pool(name="edge1", bufs=1) as pCs,
            tc.tile_pool(name="edge2", bufs=1) as pCw,
        ):
            # ---- constants ----
            # all const loads ride dedicated lane 5: they complete in the
            # first microsecond, so dep thresholds on them never queue behind
            # steady-state traffic.
            def _lane5(inst):
                if CFG["lane_const"] is not None:
                    _HW_LANES[inst.ins.name] = CFG["lane_const"]
                return inst

            # idx first: the stage-B gathers depend only on these
            idx_sb = pK.tile([P, 2 * NB], i32, tag="idx_sb", name="idx_sb")
            _lane5(nc.sync.dma_start(
                out=idx_sb[:].rearrange("p (t b) -> p t b", t=2),
                in_=idx_d[:, :].rearrange("t (b p) -> p t b", p=P),
            ))
            ii = [idx_sb[:, b:b + 1] for b in range(NB)]
            jj = [idx_sb[:, NB + b:NB + b + 1] for b in range(NB)]

            wA_sb, wC_sb = {}, {}
            for n, t_d in list(wA.items()) + list(wC.items()):
                t = pK.tile([P, 2 * C], bf16, tag=f"w_{n}", name=f"w_{n}")
                _lane5(nc.sync.dma_start(
                    out=t[:].rearrange("p (k n2) -> p k n2", k=2),
                    in_=t_d[:, :].rearrange("(k p) n2 -> p k n2", p=P),
                ))
                pair = [t[:, 0:C], t[:, C:2 * C]]
                (wA_sb if n in wA else wC_sb)[n] = pair
            lw2_t = pK.tile([P, 2], bf16, tag="lin_w2", name="lin_w2t")
            _lane5(nc.sync.dma_start(
                out=lw2_t[:].rearrange("p (k o) -> p k o", k=2),
                in_=lin_w2_d[:, :].rearrange("(k p) o -> p k o", p=P),
            ))
            lw2_sb = [lw2_t[:, 0:1], lw2_t[:, 1:2]]
            b_sb = {}
            ball = pK.tile([P, 2 * len(bnames)], f32, tag="ball", name="ball")
            _lane5(nc.sync.dma_start(out=ball[:], in_=ball_d[:, :]))
            for q, n in enumerate(bnames):
                b_sb[n] = ball[:, 2 * q:2 * q + 2]
            lb2_sb = pK.tile([1, 1], f32, tag="b_lin2", name="b_lin2")
            _lane5(nc.sync.dma_start(out=lb2_sb[:], in_=lin_b2_d[:, :]))
            beta_sb = pK.tile([P, 1], f32, tag="beta", name="beta")
            _lane5(nc.sync.dma_start(out=beta_sb[:], in_=beta_d[:, :]))

            out_row = pK.tile([1, EL], f32, tag="out_row", name="out_row")
            # natural-layout h: column = hh*N + ktg*128 + c2 encodes
            # h[pos = ktg*128 + p, channel = hh*128 + c2]; pos-space is the
            # host xT column permutation (pos ktg*128+p = node 256*(ktg//2)
            # + 2p + (ktg&1)).
            h_all = pH.tile([P, 2 * N], bf16, tag="h_all", name="h_all")
            h_view = h_all[:].rearrange("p (hh kt c) -> p hh kt c", hh=2, c=P)

            # full cnT for all 1024 edges x 8192 k, fp8:
            # col = b*8192 + tt*256 + 2e + par  <=>  k = 256*tt + 2p + par
            cnT_all = pT.tile([P, NB * N], fp8, tag="cnT", name="cnT_all")
            cnT_v = cnT_all[:].rearrange(
                "p (b tt e par) -> p b tt e par", b=NB, tt=TT, par=2)
            cnT_u16 = cnT_all[:].bitcast(u16).rearrange(
                "p (b tt e) -> p b tt e", b=NB, tt=TT)

            # ---- stage definitions ----
            xT2_tiles = {}

            def stage_a_group(g, pA, psA):
                gp, half = g // 2, g % 2
                if half == 0:
                    t = pA.tile([P, 4 * AGRP], bf16, tag="xT2",
                                name=f"xT2_{gp}")
                    ld = nc.scalar.dma_start(
                        out=t[:].rearrange("p (hh n) -> p hh n", hh=2),
                        in_=xT_d[:, gp * 2 * AGRP:(gp + 1) * 2 * AGRP]
                        .rearrange("(hh p) n -> p hh n", p=P),
                    )
                    if CFG["lane_xT"]:
                        _HW_LANES[ld.ins.name] = gp % 2
                    xT2_tiles[gp] = t
                xTv = xT2_tiles[gp][:].rearrange("p (hh n) -> p hh n", hh=2)
                off = half * AGRP
                xT = [xTv[:, 0, off:off + AGRP], xTv[:, 1, off:off + AGRP]]
                y1T = []
                for h in range(2):
                    ps = psA.tile([P, AGRP], f32, tag="psmm", name=f"psA1_{g}{h}")
                    nc.tensor.matmul(
                        ps[:], wA_sb["xlin_w1"][0][:, h * P:(h + 1) * P],
                        xT[0], start=True, stop=False,
                    )
                    nc.tensor.matmul(
                        ps[:], wA_sb["xlin_w1"][1][:, h * P:(h + 1) * P],
                        xT[1], start=False, stop=True,
                    )
                    t = pA.tile([P, AGRP], bf16, tag=f"y1T{h}", name=f"y1T{h}_{g}")
                    nc.scalar.activation(
                        t[:], ps[:], Relu, bias=b_sb["xlin_b1"][:, h:h + 1]
                    )
                    y1T.append(t)
                hTc = pA.tile([P, 2 * AGRP], bf16, tag="hT", name=f"hT_{g}")
                for h in range(2):
                    ps = psA.tile([P, AGRP], f32, tag="psmm", name=f"psA2_{g}{h}")
                    nc.tensor.matmul(
                        ps[:], wA_sb["xlin_w2"][0][:, h * P:(h + 1) * P],
                        y1T[0][:], start=True, stop=False,
                    )
                    nc.tensor.matmul(
                        ps[:], wA_sb["xlin_w2"][1][:, h * P:(h + 1) * P],
                        y1T[1][:], start=False, stop=True,
                    )
                    y2 = pA.tile([P, AGRP], bf16, tag="y2T", name=f"y2T{h}_{g}")
                    nc.scalar.activation(
                        y2[:], ps[:], Relu, bias=b_sb["xlin_b2"][:, h:h + 1]
                    )
                    nc.vector.tensor_tensor(
                        out=hTc[:, h * AGRP:(h + 1) * AGRP], in0=xT[h],
                        in1=y2[:], op=ADD
                    )
                # per-half transposes: each destination is one contiguous
                # 512-span (2-span dsts produce wrong data on this xbar)
                for h in range(2):
                    tp = nc.sync.dma_start_transpose(
                        out=h_view[:, h,
                                   g * (AGRP // P):(g + 1) * (AGRP // P), :],
                        in_=hTc[:, h * AGRP:(h + 1) * AGRP],
                    )
                    if CFG["lane_xT"]:
                        _HW_LANES[tp.ins.name] = 2 + h

            # ---- stage B pieces ----
            adj_tiles = {}

            from contextlib import nullcontext

            def _prio():
                return tc.high_priority() if CFG["hipri_adj"] else nullcontext()

            def b_gather(b):
                with _prio():
                    ai = pAdjI.tile([P, N], fp8, tag="ai", name=f"ai{b}")
                    _rr(nc.gpsimd.indirect_dma_start(
                        out=ai[:], out_offset=None, in_=adj_d[:, :],
                        in_offset=bass.IndirectOffsetOnAxis(
                            ap=ii[b][:, :1], axis=0),
                    ))
                    aj = pAdjJ.tile([P, N], fp8, tag="aj", name=f"aj{b}")
                    _rr(nc.gpsimd.indirect_dma_start(
                        out=aj[:], out_offset=None, in_=adj_d[:, :],
                        in_offset=bass.IndirectOffsetOnAxis(
                            ap=jj[b][:, :1], axis=0),
                    ))
                    adj_tiles[b] = (ai, aj)

            def b_and_xpose(b):
                with _prio():
                    ai, aj = adj_tiles.pop(b)
                    cn = ai
                    nc.vector.tensor_tensor(
                        out=cn[:].bitcast(u16), in0=ai[:].bitcast(u16),
                        in1=aj[:].bitcast(u16), op=AND,
                    )
                    tp = nc.sync.dma_start_transpose(
                        out=cnT_u16[:, b, :, :], in_=cn[:].bitcast(u16),
                    )
                    if CFG["lane_cn"] is not None:
                        _HW_LANES[tp.ins.name] = CFG["lane_cn"]

            def b_mms(g, psb):
                for tt in range(TT):
                    for par in range(2):
                        ktg = 2 * tt + par
                        for ch in range(2):
                            nc.tensor.matmul(
                                psb[ch][:],
                                h_view[:, ch, ktg, :],
                                cnT_v[:, CGRP * g:CGRP * (g + 1), tt, :, par],
                                start=(tt == 0 and par == 0),
                                stop=(tt == TT - 1 and par == 1),
                            )

            # ---- stage C ----
            prodT_map = {}

            def stage_c_prod(grp):
                blocks = range(grp * CGRP, (grp + 1) * CGRP)
                prodT = pCs.tile([P, CGRP * C], bf16, tag=f"prodT{grp}",
                                 name=f"prodT{grp}")
                prodT_map[grp] = prodT
                pc = pCs.tile([P, CGRP * C], bf16, tag=f"prodc{grp}",
                              name=f"prodc{grp}")
                for t2, b in enumerate(blocks):
                    xi = pCs.tile([P, C], bf16, tag=f"xi{b}", name=f"xi{b}")
                    gi = _rr(nc.gpsimd.indirect_dma_start(
                        out=xi[:], out_offset=None, in_=x_d[:, :],
                        in_offset=bass.IndirectOffsetOnAxis(
                            ap=ii[b][:, :1], axis=0),
                    ))
                    if CFG["sw_xixj"]:
                        _SW_LANES[gi.ins.name] = 6
                    xj = pCs.tile([P, C], bf16, tag=f"xj{b}", name=f"xj{b}")
                    gj = _rr(nc.gpsimd.indirect_dma_start(
                        out=xj[:], out_offset=None, in_=x_d[:, :],
                        in_offset=bass.IndirectOffsetOnAxis(
                            ap=jj[b][:, :1], axis=0),
                    ))
                    if CFG["sw_xixj"]:
                        _SW_LANES[gj.ins.name] = 7
                    nc.vector.tensor_tensor(
                        out=pc[:, t2 * C:(t2 + 1) * C], in0=xi[:], in1=xj[:],
                        op=MUL
                    )
                # one transpose: column blk*256 + ch*128 + c2 lands at
                # prodT[c2, blk, ch, e] - the packed mlp rhs layout
                tp = nc.sync.dma_start_transpose(
                    out=prodT[:].rearrange(
                        "p (blk hh e) -> p blk hh e", blk=CGRP, e=P),
                    in_=pc[:],
                )
                if CFG["lane_prod"] is not None:
                    _HW_LANES[tp.ins.name] = CFG["lane_prod"]

            def stage_c(grp, psb, psC, psO):
                # xcnT straight from stage-B PSUM (f32 -> bf16)
                xcnT = pCs.tile([P, 2 * W], bf16, tag=f"xcnT{grp}",
                                name=f"xcnT{grp}")
                for ch in range(2):
                    nc.vector.tensor_copy(
                        xcnT[:, ch * W:(ch + 1) * W], psb[ch][:])
                prodT = prodT_map[grp]

                def mlp_layer(rhs2, wname, bname, outtag, packed):
                    outs = []
                    for h in range(2):
                        ps = psC.tile([P, W], f32, tag="psc",
                                      name=f"psc_{grp}_{outtag}{h}")
                        if packed:
                            rhs_v = rhs2[:].rearrange(
                                "p (blk hh e) -> p blk hh e", blk=CGRP, e=P)
                            r0, r1 = rhs_v[:, :, 0, :], rhs_v[:, :, 1, :]
                        else:
                            r0, r1 = rhs2[0], rhs2[1]
                        nc.tensor.matmul(
                            ps[:], wC_sb[wname][0][:, h * P:(h + 1) * P],
                            r0, start=True, stop=False,
                        )
                        nc.tensor.matmul(
                            ps[:], wC_sb[wname][1][:, h * P:(h + 1) * P],
                            r1, start=False, stop=True,
                        )
                        t = pCw.tile([P, W], bf16, tag=f"{outtag}{h}",
                                     name=f"{outtag}{h}_{grp}")
                        nc.scalar.activation(
                            t[:], ps[:], Relu, bias=b_sb[bname][:, h:h + 1]
                        )
                        outs.append(t)
                    return outs

                xijT = mlp_layer(prodT, "xij_w", "xij_b", "xijT", True)
                u1T = mlp_layer([xcnT[:, 0:W], xcnT[:, W:2 * W]],
                                "xcn_w1", "xcn_b1", "u1T", False)
                u2T = mlp_layer([u1T[0][:], u1T[1][:]],
                                "xcn_w2", "xcn_b2", "u2T", False)
                zT = []
                for h in range(2):
                    zb = pCw.tile([P, W], bf16, tag=f"zb{h}", name=f"zb{h}_{grp}")
                    nc.vector.tensor_tensor(
                        out=zb[:], in0=u2T[h][:],
                        in1=beta_sb[:, 0:1].to_broadcast([P, W]), op=MUL,
                    )
                    zt = pCw.tile([P, W], bf16, tag=f"zT{h}", name=f"zT{h}_{grp}")
                    nc.vector.tensor_tensor(
                        out=zt[:], in0=zb[:], in1=xijT[h][:], op=ADD
                    )
                    zT.append(zt)
                vT = mlp_layer([zT[0][:], zT[1][:]], "lin_w1", "lin_b1", "vT",
                               False)
                pso = psO.tile([1, W], f32, tag="pso", name=f"pso{grp}")
                nc.tensor.matmul(
                    pso[:], lw2_sb[0][:], vT[0][:], start=True, stop=False
                )
                nc.tensor.matmul(
                    pso[:], lw2_sb[1][:], vT[1][:], start=False, stop=True
                )
                nc.scalar.activation(
                    out_row[0:1, grp * W:(grp + 1) * W], pso[:],
                    Ident, bias=lb2_sb[0:1, 0:1],
                )

            # ---- software-pipelined emission ----
            # Pool queue: all adjacency gathers first (block b+2's gather
            # SEQ-waits on AND b, so ANDs below must keep pace), then the
            # xi/xj gathers for stage C.
            with tc.tile_pool(name="psB", bufs=1, space="PSUM") as psB:
                psb = {
                    (g, ch): psB.tile([P, W], f32, tag=f"psb{g}{ch}",
                                      name=f"psb{g}{ch}")
                    for g in range(NEG) for ch in range(2)
                }

                # Paced interleave. Ring rule: AND/xpose(b) must be EMITTED
                # before gather(b+2) (aj ring) and gather(b+3) (ai ring), or
                # slot reuse silently mis-orders. ANDs 4-7 drain after stage A
                # when the DVE queue has nothing left to stall.
                and_after = {1: 0, 3: 1, 6: 2, 9: 3}
                gather_after = {2: [2], 4: [3], 7: [4], 10: [5]}
                b_gather(0)
                b_gather(1)
                with tc.tile_pool(name="stA", bufs=3) as pA, \
                     tc.tile_pool(name="psA", bufs=4, space="PSUM") as psA:
                    for g in range(NG):
                        stage_a_group(g, pA, psA)
                        if g in and_after:
                            b_and_xpose(and_after[g])
                        for b in gather_after.get(g, []):
                            b_gather(b)
                for b in range(4, NB):
                    b_and_xpose(b)
                    if b + 2 < NB:
                        b_gather(b + 2)

                with tc.tile_pool(name="psC", bufs=2, space="PSUM") as psC, \
                     tc.tile_pool(name="psO", bufs=1, space="PSUM") as psO:
                    stage_c_prod(0)
                    b_mms(0, [psb[(0, 0)], psb[(0, 1)]])
                    stage_c_prod(1)
                    stage_c(0, [psb[(0, 0)], psb[(0, 1)]], psC, psO)
                    b_mms(1, [psb[(1, 0)], psb[(1, 1)]])
                    stage_c(1, [psb[(1, 0)], psb[(1, 1)]], psC, psO)

            st = nc.sync.dma_start(out=out_d[:, :], in_=out_row[0:1, :])
            if CFG["lane_prod"] is not None:
                _HW_LANES[st.ins.name] = CFG["lane_prod"]

    return _split_multi_waits(nc) if split_waits else nc


def kernel(**inputs):
    from concourse.bass_utils import run_bass_kernel_spmd

    if "nc" not in _CACHE:
        _CACHE["nc"] = _build()
    nc = _CACHE["nc"]

    x = np.ascontiguousarray(inputs["x"], dtype=np.float32)
    adj8 = np.ascontiguousarray(inputs["adj"]).astype(ml_dtypes.float8_e4m3)
    tar = np.asarray(inputs["tar_ei"]).astype(np.int32)

    # position q = ktg*128 + p holds node 256*(ktg//2) + 2p + (ktg&1): this
    # aligns h tiles with the uint16-pair cn transpose (k = 256*tt + 2p + par
    # on partition p of k-tile tt, ktg = 2*tt + par).
    qs = np.arange(N)
    ktg, pp = qs // P, qs % P
    perm = 256 * (ktg // 2) + 2 * pp + (ktg & 1)

    def btile(b):
        return np.ascontiguousarray(np.asarray(b, dtype=np.float32).reshape(2, P).T)

    common = {
        "x": x.astype(ml_dtypes.bfloat16),
        "xTp": np.ascontiguousarray(x[perm].T).astype(ml_dtypes.bfloat16),
        "adj": adj8,
        "beta_bc": np.full((P, 1), np.asarray(inputs["beta"]).reshape(-1)[0],
                           dtype=np.float32),
        "lin_w2": np.ascontiguousarray(inputs["lin_w2"]).astype(ml_dtypes.bfloat16),
        "lin_b2": np.asarray(inputs["lin_b2"], dtype=np.float32).reshape(1, 1),
    }
    for n in ("xlin_w1", "xlin_w2", "xcn_w1", "xcn_w2", "xij_w", "lin_w1"):
        common[n] = np.ascontiguousarray(inputs[n]).astype(ml_dtypes.bfloat16)
    common["ball"] = np.ascontiguousarray(np.concatenate(
        [btile(inputs[n]) for n in
         ("xlin_b1", "xlin_b2", "xcn_b1", "xcn_b2", "xij_b", "lin_b1")],
        axis=1))

    in_maps = []
    for c in range(NCORES):
        m = dict(common)
        m["idx"] = np.ascontiguousarray(tar[:, c * EL:(c + 1) * EL])
        in_maps.append(m)

    res = run_bass_kernel_spmd(
        nc, in_maps, core_ids=list(range(NCORES)), trace=TRACE
    )
    global LAST_RESULT
    LAST_RESULT = res
    out = np.concatenate(
        [res.results[c]["out"].reshape(EL, 1) for c in range(NCORES)], axis=0
    )
    return out.astype(np.float32)


# revision 23
# speedup vs baseline: 1.1294x; 1.0972x over previous
"""CNLinkPredictor Trainium2 kernel.

Edge-sharded across 8 NeuronCores (1024 target edges each); x, adj, and the
MLP weights are replicated. Per core:
  A) h = x + MLP(x) in transposed layout: host supplies a column-PERMUTED xT
     (see below), so stage A is matmul-only on PE (bf16, N=512 moving), fused
     bias+ReLU on the scalar engine, residual on DVE, then xbar DMA-transposes
     write h back to natural layout.
  B) per 128-edge block: indirect-DMA gather of the two FULL adjacency rows
     per edge (fp8 - exact for a 0/1 adjacency), cn = ai AND aj as a bitwise
     AND on uint16 views (fp8 pairs; 0/1 fp8 codes AND exactly), one xbar
     DMA-transpose of the uint16 view (halves transpose cost vs bf16), then
     FLIPPED matmuls: lhsT = h k-tile (bf16), rhs = cnT for all 512 edges of
     an edge-group (fp8, stride-2 AP) accumulating xcnT[c,e] in PSUM.
     The uint16-pair transpose leaves k-pairs interleaved: partition p of
     k-tile tt holds k = 256*tt + 2p + par. The host permutes xT columns so
     h_view tile ktg=2*tt+par, partition p holds node 256*tt+2p+par; the
     device math is then an exact relabeling (verified vs reference).
  C) edge MLPs in transposed layout per 512-edge group; xcnT comes straight
     out of the stage-B PSUM (no transpose), xi*xj is transposed per block.

PE order A -> B(edges 0-511) -> C(0-511) -> B(512-1023) -> C(512-1023) keeps
the tensor engine hot; DVE ANDs and xbar transposes are interleaved into the
stage-A emission so the FIFO queues never head-of-line block.

Hardware pitfalls this kernel works around:
  - This walrus build accepts at most ONE sync-wait per instruction
    (_apply_tile_patch splits the Tile tail drain; _split_multi_waits hoists
    extra waits onto same-engine NoOps).
  - Concurrent 4-byte DMA traffic corrupts in-flight 2-byte xbar
    DMA-transposes, so every steady-state transfer is <= 2 bytes/element
    (fp8 adjacency, bf16/u16 everything else); the few f32/int32 loads happen
    up front and the single f32 store happens after the last transpose.
  - xbar transposes into non-contiguous destinations produce wrong data;
    all transpose targets collapse to contiguous 2-D access patterns.
"""

import numpy as np
import ml_dtypes

N = 8192
C = 256
E = 8192
NCORES = 8
EL = E // NCORES          # edges per core
P = 128
NB = EL // P              # edge blocks per core (8)
TT = N // 256             # k-pair tiles (32): tile tt holds k=256*tt+2p+par
AGRP = 512                # stage-A node group
NG = N // AGRP            # stage-A groups (16)
CGRP = 4                  # blocks per edge-group (512 edges)
NEG = NB // CGRP          # edge groups (2)
W = CGRP * P              # edges per group (512)

_CACHE = {}
TRACE = False
LAST_RESULT = None
_HW_LANES = {}            # instruction name -> pinned DMAHW sem lane
_SW_LANES = {}            # instruction name -> pinned DMASW sem lane
CFG = {
    "lane_cn": 7,          # DMAHW lane for cn transposes (None = default rr)
    "lane_const": None,    # lane for const loads
    "lane_prod": None,     # lane for prodT xposes + out store
    "lane_xT": False,      # xT loads on lanes 0/1, hT xposes on 2/3
    "sw_xixj": False,      # xi/xj on SW lanes 6/7
    "hipri_adj": True,    # high_priority on gathers + and/xpose
    "inplace_and": True,   # AND writes into ai tile (frees cn pool)
}


def _apply_tile_patch():
    """Split the Tile tail-drain's multi-sem wait onto individual SP nops."""
    from concourse.tile import TileContext
    from concourse.vector_clock import ScopedClock

    if getattr(TileContext, "_drain_patched", False):
        return

    def _patched(self, tick_clock, wait_clock):
        nc = self.nc
        collector = nc.sync.nop()
        wait_clock.add_sem_waits(
            collector.ins, ScopedClock({None: tick_clock.global_clock})
        )
        si = collector.ins.sync_info
        waits = list(si.on_wait) if si is not None and si.on_wait else []
        if si is not None and len(waits) > 1:
            name_to_handle = {h.name: h for h in self.sems.allocated().values()}
            si.on_wait = [waits[0]]
            for w in waits[1:]:
                op = {
                    "sem-ge-imm": "sem-ge",
                    "sem-eq-imm": "sem-eq",
                    "sem-le-imm": "sem-le",
                }.get(str(w.wait_mode), "sem-ge")
                nc.sync.nop().wait_op(name_to_handle[w.ant_name], w.wait_value, op)
        nc.sync.drain()
        nc.all_engine_barrier()
        assert self.sems is not None
        popped = nc._tile_sem_poison_stack.pop()
        assert popped is self._sem_poison
        nc.clear_and_free_semaphores(list(self.sems.allocated().values()))
        nc.all_engine_barrier()

    TileContext._drain_and_barrier = _patched
    TileContext._drain_patched = True


def _apply_lane_patch():
    """Honor a `_hw_lane` attribute on DMA instructions: pin them to that
    DMAHW sem lane instead of the global round-robin, and keep the round-robin
    off the reserved lanes. Without this, slow gather-gated transposes share a
    lane counter with fast stage-A DMAs and unrelated deps summarize onto the
    slow stream (multi-10us head-of-line stalls)."""
    from concourse.tile_sem_assignment import TileClockTick

    if getattr(TileClockTick, "_lane_patched", False):
        return
    orig = TileClockTick._assign_tick

    def _patched(self, inst):
        name = getattr(inst, "name", None)
        hw = _HW_LANES.get(name)
        if hw is not None:
            saved = self.next_hw_dma_idx
            self.next_hw_dma_idx = hw
            orig(self, inst)
            self.next_hw_dma_idx = saved
            return
        sw = _SW_LANES.get(name)
        if sw is not None:
            saved = self.next_sw_dma_idx
            self.next_sw_dma_idx = sw
            orig(self, inst)
            self.next_sw_dma_idx = saved
            return
        orig(self, inst)
        rhw = set(_HW_LANES.values())
        rsw = set(_SW_LANES.values())
        if len(rhw) < 8:
            while self.next_hw_dma_idx in rhw:
                self.next_hw_dma_idx = (self.next_hw_dma_idx + 1) % 8
        if len(rsw) < self.swdge_sem_count:
            while self.next_sw_dma_idx in rsw:
                self.next_sw_dma_idx = (self.next_sw_dma_idx + 1) % self.swdge_sem_count

    TileClockTick._assign_tick = _patched
    TileClockTick._lane_patched = True


def _split_multi_waits(nc):
    """Hoist extra sync-waits onto same-engine NoOps (sequential waits ==
    ANDed waits); this walrus build allows one wait per instruction."""
    import concourse.mybir as mybir

    cnt = 0
    for fn in nc.m.functions:
        for bb in fn.blocks:
            out = []
            for inst in bb.instructions:
                si = getattr(inst, "sync_info", None)
                waits = list(si.on_wait) if si is not None and si.on_wait else []
                if len(waits) > 1:
                    for w in waits[:-1]:
                        nop = mybir.InstNoOp(name=f"ws-{cnt}", ins=[], outs=[])
                        cnt += 1
                        nop.engine = inst.engine
                        nop.sync_info = mybir.SyncInfo(on_wait=[w], on_update=[])
                        out.append(nop)
                    si.on_wait = [waits[-1]]
                out.append(inst)
            bb.instructions = out
    return nc


def _build(split_waits=True):
    import concourse.bass as bass
    import concourse.mybir as mybir
    from concourse.tile import TileContext

    _apply_tile_patch()
    _apply_lane_patch()
    _HW_LANES.clear()
    _SW_LANES.clear()

    f32 = mybir.dt.float32
    bf16 = mybir.dt.bfloat16
    fp8 = mybir.dt.float8e4
    u16 = mybir.dt.uint16
    i32 = mybir.dt.int32
    Relu = mybir.ActivationFunctionType.Relu
    Ident = mybir.ActivationFunctionType.Identity
    MUL = mybir.AluOpType.mult
    ADD = mybir.AluOpType.add
    AND = mybir.AluOpType.bitwise_and

    nc = bass.Bass(num_swdge_queues=4)

    xT_d = nc.dram_tensor("xTp", [C, N], bf16, kind="ExternalInput")
    x_d = nc.dram_tensor("x", [N, C], bf16, kind="ExternalInput")
    adj_d = nc.dram_tensor("adj", [N, N], fp8, kind="ExternalInput")
    idx_d = nc.dram_tensor("idx", [2, EL], i32, kind="ExternalInput")
    wA = {n: nc.dram_tensor(n, [C, C], bf16, kind="ExternalInput")
          for n in ("xlin_w1", "xlin_w2")}
    wC = {n: nc.dram_tensor(n, [C, C], bf16, kind="ExternalInput")
          for n in ("xcn_w1", "xcn_w2", "xij_w", "lin_w1")}
    lin_w2_d = nc.dram_tensor("lin_w2", [C, 1], bf16, kind="ExternalInput")
    bnames = ["xlin_b1", "xlin_b2", "xcn_b1", "xcn_b2", "xij_b", "lin_b1"]
    ball_d = nc.dram_tensor("ball", [P, 2 * len(bnames)], f32,
                            kind="ExternalInput")
    lin_b2_d = nc.dram_tensor("lin_b2", [1, 1], f32, kind="ExternalInput")
    beta_d = nc.dram_tensor("beta_bc", [P, 1], f32, kind="ExternalInput")
    out_d = nc.dram_tensor("out", [1, EL], f32, kind="ExternalOutput")

    _swq = [0]

    def _rr(inst):
        q = _swq[0] % 4
        _swq[0] += 1
        if q:
            inst.ins.queue = f"qPoolDynamic{q}"
        return inst

    with TileContext(nc) as tc:
        with (
            tc.tile_pool(name="const", bufs=1) as pK,
            tc.tile_pool(name="hpool", bufs=1) as pH,
            tc.tile_pool(name="adji", bufs=3) as pAdjI,
            tc.tile_pool(name="adjj", bufs=2) as pAdjJ,
            tc.tile_pool(name="cnT", bufs=1) as pT,
            tc.tile_pool(name="edge1", bufs=1) as pCs,
            tc.tile_pool(name="edge2", bufs=1) as pCw,
        ):
            # ---- constants ----
            # all const loads ride dedicated lane 5: they complete in the
            # first microsecond, so dep thresholds on them never queue behind
            # steady-state traffic.
            def _lane5(inst):
                if CFG["lane_const"] is not None:
                    _HW_LANES[inst.ins.name] = CFG["lane_const"]
                return inst

            # idx first: the stage-B gathers depend only on these
            idx_sb = pK.tile([P, 2 * NB], i32, tag="idx_sb", name="idx_sb")
            _lane5(nc.sync.dma_start(
                out=idx_sb[:].rearrange("p (t b) -> p t b", t=2),
                in_=idx_d[:, :].rearrange("t (b p) -> p t b", p=P),
            ))
            ii = [idx_sb[:, b:b + 1] for b in range(NB)]
            jj = [idx_sb[:, NB + b:NB + b + 1] for b in range(NB)]

            wA_sb, wC_sb = {}, {}
            for n, t_d in list(wA.items()) + list(wC.items()):
                t = pK.tile([P, 2 * C], bf16, tag=f"w_{n}", name=f"w_{n}")
                _lane5(nc.sync.dma_start(
                    out=t[:].rearrange("p (k n2) -> p k n2", k=2),
                    in_=t_d[:, :].rearrange("(k p) n2 -> p k n2", p=P),
                ))
                pair = [t[:, 0:C], t[:, C:2 * C]]
                (wA_sb if n in wA else wC_sb)[n] = pair
            lw2_t = pK.tile([P, 2], bf16, tag="lin_w2", name="lin_w2t")
            _lane5(nc.sync.dma_start(
                out=lw2_t[:].rearrange("p (k o) -> p k o", k=2),
                in_=lin_w2_d[:, :].rearrange("(k p) o -> p k o", p=P),
            ))
            lw2_sb = [lw2_t[:, 0:1], lw2_t[:, 1:2]]
            b_sb = {}
            ball = pK.tile([P, 2 * len(bnames)], f32, tag="ball", name="ball")
            _lane5(nc.sync.dma_start(out=ball[:], in_=ball_d[:, :]))
            for q, n in enumerate(bnames):
                b_sb[n] = ball[:, 2 * q:2 * q + 2]
            lb2_sb = pK.tile([1, 1], f32, tag="b_lin2", name="b_lin2")
            _lane5(nc.sync.dma_start(out=lb2_sb[:], in_=lin_b2_d[:, :]))
            beta_sb = pK.tile([P, 1], f32, tag="beta", name="beta")
            _lane5(nc.sync.dma_start(out=beta_sb[:], in_=beta_d[:, :]))

            out_row = pK.tile([1, EL], f32, tag="out_row", name="out_row")
            # natural-layout h: column = hh*N + ktg*128 + c2 encodes
            # h[pos = ktg*128 + p, channel = hh*128 + c2]; pos-space is the
            # host xT column permutation (pos ktg*128+p = node 256*(ktg//2)
            # + 2p + (ktg&1)).
            h_all = pH.tile([P, 2 * N], bf16, tag="h_all", name="h_all")
            h_view = h_all[:].rearrange("p (hh kt c) -> p hh kt c", hh=2, c=P)

            # full cnT for all 1024 edges x 8192 k, fp8:
            # col = b*8192 + tt*256 + 2e + par  <=>  k = 256*tt + 2p + par
            cnT_all = pT.tile([P, NB * N], fp8, tag="cnT", name="cnT_all")
            cnT_v = cnT_all[:].rearrange(
                "p (b tt e par) -> p b tt e par", b=NB, tt=TT, par=2)
            cnT_u16 = cnT_all[:].bitcast(u16).rearrange(
                "p (b tt e) -> p b tt e", b=NB, tt=TT)

            # ---- stage definitions ----
            def stage_a_group(g, pA, psA):
                m0 = g * AGRP
                xT = []
                for h in range(2):
                    t = pA.tile([P, AGRP], bf16, tag=f"xT{h}", name=f"xT{h}_{g}")
                    ld = nc.scalar.dma_start(
                        out=t[:], in_=xT_d[h * P:(h + 1) * P, m0:m0 + AGRP]
                    )
                    if CFG["lane_xT"]:
                        _HW_LANES[ld.ins.name] = h
                    xT.append(t[:])
                y1T = []
                for h in range(2):
                    ps = psA.tile([P, AGRP], f32, tag="psmm", name=f"psA1_{g}{h}")
                    nc.tensor.matmul(
                        ps[:], wA_sb["xlin_w1"][0][:, h * P:(h + 1) * P],
                        xT[0], start=True, stop=False,
                    )
                    nc.tensor.matmul(
                        ps[:], wA_sb["xlin_w1"][1][:, h * P:(h + 1) * P],
                        xT[1], start=False, stop=True,
                    )
                    t = pA.tile([P, AGRP], bf16, tag=f"y1T{h}", name=f"y1T{h}_{g}")
                    nc.scalar.activation(
                        t[:], ps[:], Relu, bias=b_sb["xlin_b1"][:, h:h + 1]
                    )
                    y1T.append(t)
                hTc = pA.tile([P, 2 * AGRP], bf16, tag="hT", name=f"hT_{g}")
                for h in range(2):
                    ps = psA.tile([P, AGRP], f32, tag="psmm", name=f"psA2_{g}{h}")
                    nc.tensor.matmul(
                        ps[:], wA_sb["xlin_w2"][0][:, h * P:(h + 1) * P],
                        y1T[0][:], start=True, stop=False,
                    )
                    nc.tensor.matmul(
                        ps[:], wA_sb["xlin_w2"][1][:, h * P:(h + 1) * P],
                        y1T[1][:], start=False, stop=True,
                    )
                    y2 = pA.tile([P, AGRP], bf16, tag="y2T", name=f"y2T{h}_{g}")
                    nc.scalar.activation(
                        y2[:], ps[:], Relu, bias=b_sb["xlin_b2"][:, h:h + 1]
                    )
                    nc.vector.tensor_tensor(
                        out=hTc[:, h * AGRP:(h + 1) * AGRP], in0=xT[h],
                        in1=y2[:], op=ADD
                    )
                # per-half transposes: each destination is one contiguous
                # 512-span (2-span dsts produce wrong data on this xbar)
                for h in range(2):
                    tp = nc.sync.dma_start_transpose(
                        out=h_view[:, h,
                                   g * (AGRP // P):(g + 1) * (AGRP // P), :],
                        in_=hTc[:, h * AGRP:(h + 1) * AGRP],
                    )
                    if CFG["lane_xT"]:
                        _HW_LANES[tp.ins.name] = 2 + h

            # ---- stage B pieces ----
            adj_tiles = {}

            from contextlib import nullcontext

            def _prio():
                return tc.high_priority() if CFG["hipri_adj"] else nullcontext()

            def b_gather(b):
                with _prio():
                    ai = pAdjI.tile([P, N], fp8, tag="ai", name=f"ai{b}")
                    _rr(nc.gpsimd.indirect_dma_start(
                        out=ai[:], out_offset=None, in_=adj_d[:, :],
                        in_offset=bass.IndirectOffsetOnAxis(
                            ap=ii[b][:, :1], axis=0),
                    ))
                    aj = pAdjJ.tile([P, N], fp8, tag="aj", name=f"aj{b}")
                    _rr(nc.gpsimd.indirect_dma_start(
                        out=aj[:], out_offset=None, in_=adj_d[:, :],
                        in_offset=bass.IndirectOffsetOnAxis(
                            ap=jj[b][:, :1], axis=0),
                    ))
                    adj_tiles[b] = (ai, aj)

            def b_and_xpose(b):
                with _prio():
                    ai, aj = adj_tiles.pop(b)
                    cn = ai
                    nc.vector.tensor_tensor(
                        out=cn[:].bitcast(u16), in0=ai[:].bitcast(u16),
                        in1=aj[:].bitcast(u16), op=AND,
                    )
                    tp = nc.sync.dma_start_transpose(
                        out=cnT_u16[:, b, :, :], in_=cn[:].bitcast(u16),
                    )
                    if CFG["lane_cn"] is not None:
                        _HW_LANES[tp.ins.name] = CFG["lane_cn"]

            def b_mms(g, psb):
                for tt in range(TT):
                    for par in range(2):
                        ktg = 2 * tt + par
                        for ch in range(2):
                            nc.tensor.matmul(
                                psb[ch][:],
                                h_view[:, ch, ktg, :],
                                cnT_v[:, CGRP * g:CGRP * (g + 1), tt, :, par],
                                start=(tt == 0 and par == 0),
                                stop=(tt == TT - 1 and par == 1),
                            )

            # ---- stage C ----
            prodT_map = {}

            def stage_c_prod(grp):
                blocks = range(grp * CGRP, (grp + 1) * CGRP)
                prodT = pCs.tile([P, CGRP * C], bf16, tag=f"prodT{grp}",
                                 name=f"prodT{grp}")
                prodT_map[grp] = prodT
                prodT_v = prodT[:].rearrange(
                    "p (blk hh e) -> p blk hh e", blk=CGRP, e=P)
                for t2, b in enumerate(blocks):
                    xi = pCs.tile([P, C], bf16, tag=f"xi{b}", name=f"xi{b}")
                    gi = _rr(nc.gpsimd.indirect_dma_start(
                        out=xi[:], out_offset=None, in_=x_d[:, :],
                        in_offset=bass.IndirectOffsetOnAxis(
                            ap=ii[b][:, :1], axis=0),
                    ))
                    if CFG["sw_xixj"]:
                        _SW_LANES[gi.ins.name] = 6
                    xj = pCs.tile([P, C], bf16, tag=f"xj{b}", name=f"xj{b}")
                    gj = _rr(nc.gpsimd.indirect_dma_start(
                        out=xj[:], out_offset=None, in_=x_d[:, :],
                        in_offset=bass.IndirectOffsetOnAxis(
                            ap=jj[b][:, :1], axis=0),
                    ))
                    if CFG["sw_xixj"]:
                        _SW_LANES[gj.ins.name] = 7
                    pt = pCw.tile([P, C], bf16, tag=f"prod{b % 2}",
                                  name=f"prod{b}")
                    nc.vector.tensor_tensor(
                        out=pt[:], in0=xi[:], in1=xj[:], op=MUL
                    )
                    tp = nc.sync.dma_start_transpose(
                        out=prodT_v[:, t2, :, :], in_=pt[:],
                    )
                    if CFG["lane_prod"] is not None:
                        _HW_LANES[tp.ins.name] = CFG["lane_prod"]

            def stage_c(grp, psb, psC, psO):
                # xcnT straight from stage-B PSUM (f32 -> bf16)
                xcnT = pCs.tile([P, 2 * W], bf16, tag=f"xcnT{grp}",
                                name=f"xcnT{grp}")
                for ch in range(2):
                    nc.vector.tensor_copy(
                        xcnT[:, ch * W:(ch + 1) * W], psb[ch][:])
                prodT = prodT_map[grp]

                def mlp_layer(rhs2, wname, bname, outtag, packed):
                    outs = []
                    for h in range(2):
                        ps = psC.tile([P, W], f32, tag="psc",
                                      name=f"psc_{grp}_{outtag}{h}")
                        if packed:
                            rhs_v = rhs2[:].rearrange(
                                "p (blk hh e) -> p blk hh e", blk=CGRP, e=P)
                            r0, r1 = rhs_v[:, :, 0, :], rhs_v[:, :, 1, :]
                        else:
                            r0, r1 = rhs2[0], rhs2[1]
                        nc.tensor.matmul(
                            ps[:], wC_sb[wname][0][:, h * P:(h + 1) * P],
                            r0, start=True, stop=False,
                        )
                        nc.tensor.matmul(
                            ps[:], wC_sb[wname][1][:, h * P:(h + 1) * P],
                            r1, start=False, stop=True,
                        )
                        t = pCw.tile([P, W], bf16, tag=f"{outtag}{h}",
                                     name=f"{outtag}{h}_{grp}")
                        nc.scalar.activation(
                            t[:], ps[:], Relu, bias=b_sb[bname][:, h:h + 1]
                        )
                        outs.append(t)
                    return outs

                xijT = mlp_layer(prodT, "xij_w", "xij_b", "xijT", True)
                u1T = mlp_layer([xcnT[:, 0:W], xcnT[:, W:2 * W]],
                                "xcn_w1", "xcn_b1", "u1T", False)
                u2T = mlp_layer([u1T[0][:], u1T[1][:]],
                                "xcn_w2", "xcn_b2", "u2T", False)
                zT = []
                for h in range(2):
                    zb = pCw.tile([P, W], bf16, tag=f"zb{h}", name=f"zb{h}_{grp}")
                    nc.vector.tensor_tensor(
                        out=zb[:], in0=u2T[h][:],
                        in1=beta_sb[:, 0:1].to_broadcast([P, W]), op=MUL,
                    )
                    zt = pCw.tile([P, W], bf16, tag=f"zT{h}", name=f"zT{h}_{grp}")
                    nc.vector.tensor_tensor(
                        out=zt[:], in0=zb[:], in1=xijT[h][:], op=ADD
                    )
                    zT.append(zt)
                vT = mlp_layer([zT[0][:], zT[1][:]], "lin_w1", "lin_b1", "vT",
                               False)
                pso = psO.tile([1, W], f32, tag="pso", name=f"pso{grp}")
                nc.tensor.matmul(
                    pso[:], lw2_sb[0][:], vT[0][:], start=True, stop=False
                )
                nc.tensor.matmul(
                    pso[:], lw2_sb[1][:], vT[1][:], start=False, stop=True
                )
                nc.scalar.activation(
                    out_row[0:1, grp * W:(grp + 1) * W], pso[:],
                    Ident, bias=lb2_sb[0:1, 0:1],
                )

            # ---- software-pipelined emission ----
            # Pool queue: all adjacency gathers first (block b+2's gather
            # SEQ-waits on AND b, so ANDs below must keep pace), then the
            # xi/xj gathers for stage C.
            with tc.tile_pool(name="psB", bufs=1, space="PSUM") as psB:
                psb = {
                    (g, ch): psB.tile([P, W], f32, tag=f"psb{g}{ch}",
                                      name=f"psb{g}{ch}")
                    for g in range(NEG) for ch in range(2)
                }

                # Paced interleave. Ring rule: AND/xpose(b) must be EMITTED
                # before gather(b+2) (aj ring) and gather(b+3) (ai ring), or
                # slot reuse silently mis-orders. ANDs 4-7 drain after stage A
                # when the DVE queue has nothing left to stall.
                and_after = {1: 0, 3: 1, 6: 2, 9: 3}
                gather_after = {2: [2], 4: [3], 7: [4], 10: [5]}
                b_gather(0)
                b_gather(1)
                with tc.tile_pool(name="stA", bufs=3) as pA, \
                     tc.tile_pool(name="psA", bufs=4, space="PSUM") as psA:
                    for g in range(NG):
                        stage_a_group(g, pA, psA)
                        if g in and_after:
                            b_and_xpose(and_after[g])
                        for b in gather_after.get(g, []):
                            b_gather(b)
                for b in range(4, NB):
                    b_and_xpose(b)
                    if b + 2 < NB:
                        b_gather(b + 2)

                with tc.tile_pool(name="psC", bufs=2, space="PSUM") as psC, \
                     tc.tile_pool(name="psO", bufs=1, space="PSUM") as psO:
                    stage_c_prod(0)
                    b_mms(0, [psb[(0, 0)], psb[(0, 1)]])
                    stage_c_prod(1)
                    stage_c(0, [psb[(0, 0)], psb[(0, 1)]], psC, psO)
                    b_mms(1, [psb[(1, 0)], psb[(1, 1)]])
                    stage_c(1, [psb[(1, 0)], psb[(1, 1)]], psC, psO)

            st = nc.sync.dma_start(out=out_d[:, :], in_=out_row[0:1, :])
            if CFG["lane_prod"] is not None:
                _HW_LANES[st.ins.name] = CFG["lane_prod"]

    return _split_multi_waits(nc) if split_waits else nc


def kernel(**inputs):
    from concourse.bass_utils import run_bass_kernel_spmd

    if "nc" not in _CACHE:
        _CACHE["nc"] = _build()
    nc = _CACHE["nc"]

    x = np.ascontiguousarray(inputs["x"], dtype=np.float32)
    adj8 = np.ascontiguousarray(inputs["adj"]).astype(ml_dtypes.float8_e4m3)
    tar = np.asarray(inputs["tar_ei"]).astype(np.int32)

    # position q = ktg*128 + p holds node 256*(ktg//2) + 2p + (ktg&1): this
    # aligns h tiles with the uint16-pair cn transpose (k = 256*tt + 2p + par
    # on partition p of k-tile tt, ktg = 2*tt + par).
    qs = np.arange(N)
    ktg, pp = qs // P, qs % P
    perm = 256 * (ktg // 2) + 2 * pp + (ktg & 1)

    def btile(b):
        return np.ascontiguousarray(np.asarray(b, dtype=np.float32).reshape(2, P).T)

    common = {
        "x": x.astype(ml_dtypes.bfloat16),
        "xTp": np.ascontiguousarray(x[perm].T).astype(ml_dtypes.bfloat16),
        "adj": adj8,
        "beta_bc": np.full((P, 1), np.asarray(inputs["beta"]).reshape(-1)[0],
                           dtype=np.float32),
        "lin_w2": np.ascontiguousarray(inputs["lin_w2"]).astype(ml_dtypes.bfloat16),
        "lin_b2": np.asarray(inputs["lin_b2"], dtype=np.float32).reshape(1, 1),
    }
    for n in ("xlin_w1", "xlin_w2", "xcn_w1", "xcn_w2", "xij_w", "lin_w1"):
        common[n] = np.ascontiguousarray(inputs[n]).astype(ml_dtypes.bfloat16)
    common["ball"] = np.ascontiguousarray(np.concatenate(
        [btile(inputs[n]) for n in
         ("xlin_b1", "xlin_b2", "xcn_b1", "xcn_b2", "xij_b", "lin_b1")],
        axis=1))

    in_maps = []
    for c in range(NCORES):
        m = dict(common)
        m["idx"] = np.ascontiguousarray(tar[:, c * EL:(c + 1) * EL])
        in_maps.append(m)

    res = run_bass_kernel_spmd(
        nc, in_maps, core_ids=list(range(NCORES)), trace=TRACE
    )
    global LAST_RESULT
    LAST_RESULT = res
    out = np.concatenate(
        [res.results[c]["out"].reshape(EL, 1) for c in range(NCORES)], axis=0
    )
    return out.astype(np.float32)
